# revision 8
# baseline (speedup 1.0000x reference)
"""Binarized MLP (784 -> 1024 -> 1024 -> 1024 -> 10) on 8 TRN2 NeuronCores.

Data-parallel over the batch (16384 rows -> 2048 per core), weights replicated.

Math notes (these make the kernel both fast and numerically faithful):
  * Layers 1-2 outputs are only ever consumed through binarize(hardtanh(bn(h))).
    Since hardtanh preserves sign and bn here is (h - m) * rsqrt(v+eps) * g + be
    with g > 0, be == 0, the next-layer input is exactly sign(h + (b - m)).
    That is one ScalarE Sign activation with a per-partition bias, no bn needed.
  * fc2/fc3 multiply two +-1 operands -> exact in fp8(e4m3) with fp32 PSUM
    accumulation (integer partial sums, magnitude <= 1024). DoubleRow perf mode
    contracts two 128-row chunks per pass (2 fp8 weights per PE cell).
  * fc1 keeps x at full precision via an exact fp16 hi/lo split:
    x = hi + lo with hi = fp16(x), lo = fp16(x - hi); products with +-1 weights
    are exact, so accuracy ~ fp32 matmul, at 2 bf16-rate passes.
  * fc4 + log_softmax: logits computed feature-major [10, B], PE-transposed to
    [B, 10]; log_softmax without max-subtraction (logits are small; exp is safe).

Loop order: weights stationary per (m, k); all 4 batch column chunks stream
per weight load (amortizes LDWEIGHTS). 4 PSUM banks accumulate per m-tile,
8-slot pool double-buffers across m-tiles.
"""

import os
import numpy as np

N_CORES = 8
B_FULL = 16384
BS = B_FULL // N_CORES  # 2048 rows per core
IN_F = 784
K1C = 7                 # s1 weight chunks of 128 (784 padded to 896; chunk 6 = packed tail)
XC = 13                 # fc1 x chunks: 6 hi + 6 lo + 1 packed hi/lo tail
H = 1024
HC = 8                  # hidden chunks of 128
OUT_F = 10
NSPLIT = 4              # batch column chunks of 512
NB = BS // NSPLIT       # 512
BT = BS // 128          # 16 batch tiles of 128 for the output transpose

LAST_RESULT = None      # BassKernelResults of the most recent run (for test.py)

_PLAN = {}


def _build_nc():
    import concourse.bass as bass
    import concourse.mybir as mybir
    import concourse.tile as tile
    from concourse import bacc
    from concourse.bass import ts
    from concourse.masks import make_identity

    f32 = mybir.dt.float32
    f16 = mybir.dt.float16
    f8 = mybir.dt.float8e4
    AF = mybir.ActivationFunctionType
    ALU = mybir.AluOpType
    DR = mybir.MatmulPerfMode.DoubleRow

    nc = bacc.Bacc(None)

    x_t = nc.dram_tensor("xc", [XC, 128, BS], f16, kind="ExternalInput")
    s1_t = nc.dram_tensor("s1t", [HC, K1C, 128, 128], f16, kind="ExternalInput")
    s2_t = nc.dram_tensor("s2t", [HC, HC, 128, 128], f8, kind="ExternalInput")
    s3_t = nc.dram_tensor("s3t", [HC, HC, 128, 128], f8, kind="ExternalInput")
    w4_t = nc.dram_tensor("w4t", [HC, 128, OUT_F], f16, kind="ExternalInput")
    b1_t = nc.dram_tensor("bias1", [H], f32, kind="ExternalInput")
    b2_t = nc.dram_tensor("bias2", [H], f32, kind="ExternalInput")
    sc3_t = nc.dram_tensor("sc3", [H], f32, kind="ExternalInput")
    sh3_t = nc.dram_tensor("sh3", [H], f32, kind="ExternalInput")
    b4_t = nc.dram_tensor("b4", [OUT_F], f32, kind="ExternalInput")
    y_t = nc.dram_tensor("y", [BS, OUT_F], f32, kind="ExternalOutput")

    with tile.TileContext(nc) as tc:
        with (
            tc.tile_pool(name="consts", bufs=1) as consts,
            tc.tile_pool(name="tmp", bufs=4) as tmp,
            tc.tile_pool(name="psum", bufs=8, space="PSUM") as psum,
        ):
            x_sb = consts.tile([128, XC, BS], f16, tag="xc")
            s1_sb = consts.tile([128, HC, K1C, 128], f16, tag="s1")
            s2_sb = consts.tile([128, HC, HC, 128], f8, tag="s2")
            s3_sb = consts.tile([128, HC, HC, 128], f8, tag="s3")
            w4_sb = consts.tile([128, HC, OUT_F], f16, tag="w4")
            b1v = consts.tile([128, HC], f32, tag="b1v")
            b2v = consts.tile([128, HC], f32, tag="b2v")
            sc3v = consts.tile([128, HC], f32, tag="sc3v")
            sh3v = consts.tile([128, HC], f32, tag="sh3v")
            b4bc = consts.tile([128, OUT_F], f32, tag="b4bc")
            ident = consts.tile([OUT_F, OUT_F], f32, tag="ident")
            act1 = consts.tile([128, HC, BS], f8, tag="act1")
            act2 = consts.tile([128, HC, BS], f8, tag="act2")
            act3 = consts.tile([128, HC, BS], f16, tag="act3")
            logits = consts.tile([OUT_F, BS], f32, tag="logits")
            lt = consts.tile([128, BT, OUT_F], f32, tag="lt")
            esb = consts.tile([128, BT, OUT_F], f32, tag="esb")
            lse = consts.tile([128, BT], f32, tag="lse")
            outf = consts.tile([128, BT, OUT_F], f32, tag="outf")

            # ---- input DMAs: weights for m=0 first, x chunks spread over
            # four issuing engines so enqueue parallelizes ----
            nc.sync.dma_start(out=s1_sb[:, 0], in_=s1_t[0].rearrange("k p c -> p k c"))
            nc.gpsimd.dma_start(out=b1v, in_=b1_t[:].rearrange("(m p) -> p m", p=128))
            dma_engs = [nc.gpsimd, nc.scalar, nc.sync]
            for k in range(XC):
                dma_engs[k % 3].dma_start(out=x_sb[:, k], in_=x_t[k])
            for m in range(1, HC):
                nc.sync.dma_start(
                    out=s1_sb[:, m], in_=s1_t[m].rearrange("k p c -> p k c")
                )

            # ---- fc1: h1 = xT.T @ s1T (feature-major), sign -> act1 ----
            # x chunks: 0-5 = hi rows 0-767, 6-11 = lo rows 0-767,
            # 12 = packed tail (hi rows 768-783 @p0-15, lo @p32-47).
            for m in range(HC):
                pss = [psum.tile([128, NB], f32, tag="mm", name="ps") for _ in range(NSPLIT)]
                for k in range(XC):
                    wk = k if k < 6 else (k - 6 if k < 12 else 6)
                    for n in range(NSPLIT):
                        nc.tensor.matmul(
                            pss[n], s1_sb[:, m, wk], x_sb[:, k, ts(n, NB)],
                            start=(k == 0), stop=(k == XC - 1),
                        )
                for n in range(NSPLIT):
                    nc.scalar.activation(
                        act1[:, m, ts(n, NB)], pss[n], AF.Sign, bias=b1v[:, m:m + 1]
                    )

            # later-layer weights (scheduler overlaps these DMAs with fc1)
            for m in range(HC):
                nc.gpsimd.dma_start(
                    out=s2_sb[:, m], in_=s2_t[m].rearrange("k p c -> p k c")
                )
            nc.sync.dma_start(out=b2v, in_=b2_t[:].rearrange("(m p) -> p m", p=128))
            for m in range(HC):
                nc.gpsimd.dma_start(
                    out=s3_sb[:, m], in_=s3_t[m].rearrange("k p c -> p k c")
                )
            nc.sync.dma_start(out=sc3v, in_=sc3_t[:].rearrange("(m p) -> p m", p=128))
            nc.sync.dma_start(out=sh3v, in_=sh3_t[:].rearrange("(m p) -> p m", p=128))
            nc.sync.dma_start(out=w4_sb, in_=w4_t.rearrange("k p o -> p k o"))
            b4_ap = b4_t[:]
            nc.sync.dma_start(
                out=b4bc,
                in_=bass.AP(tensor=b4_ap.tensor, offset=b4_ap.offset,
                            ap=[[0, 128]] + list(b4_ap.ap)),
            )
            make_identity(nc, ident)

            # ---- fc2: binary x binary, fp8 DoubleRow, sign -> act2 ----
            for m in range(HC):
                pss = [psum.tile([128, NB], f32, tag="mm", name="ps") for _ in range(NSPLIT)]
                for kk in range(HC // 2):
                    ksl = slice(2 * kk, 2 * kk + 2)
                    for n in range(NSPLIT):
                        nc.tensor.matmul(
                            pss[n], s2_sb[:, m, ksl], act1[:, ksl, ts(n, NB)],
                            start=(kk == 0), stop=(kk == HC // 2 - 1),
                            perf_mode=DR,
                        )
                for n in range(NSPLIT):
                    nc.scalar.activation(
                        act2[:, m, ts(n, NB)], pss[n], AF.Sign, bias=b2v[:, m:m + 1]
                    )

            # ---- fc3: fp8 DoubleRow, bn affine + hardtanh -> act3 (DVE) ----
            for m in range(HC):
                pss = [psum.tile([128, NB], f32, tag="mm", name="ps") for _ in range(NSPLIT)]
                for kk in range(HC // 2):
                    ksl = slice(2 * kk, 2 * kk + 2)
                    for n in range(NSPLIT):
                        nc.tensor.matmul(
                            pss[n], s3_sb[:, m, ksl], act2[:, ksl, ts(n, NB)],
                            start=(kk == 0), stop=(kk == HC // 2 - 1),
                            perf_mode=DR,
                        )
                for n in range(NSPLIT):
                    t = tmp.tile([128, NB], f32, tag="t3")
                    nc.vector.tensor_scalar(
                        out=t, in0=pss[n],
                        scalar1=sc3v[:, m:m + 1], scalar2=sh3v[:, m:m + 1],
                        op0=ALU.mult, op1=ALU.add,
                    )
                    nc.vector.tensor_scalar(
                        out=act3[:, m, ts(n, NB)], in0=t,
                        scalar1=-1.0, scalar2=1.0,
                        op0=ALU.max, op1=ALU.min,
                    )

            # ---- fc4: logits[10, BS]; per-n copy + transpose pipelined ----
            for n in range(NSPLIT):
                ps4 = psum.tile([OUT_F, NB], f32, tag="mm", name="ps4")
                for k in range(HC):
                    nc.tensor.matmul(
                        ps4, w4_sb[:, k], act3[:, k, ts(n, NB)],
                        start=(k == 0), stop=(k == HC - 1),
                    )
                nc.scalar.copy(logits[:, ts(n, NB)], ps4)
                for i in range(4 * n, 4 * n + 4):
                    pt = psum.tile([128, OUT_F], f32, tag="mm")
                    nc.tensor.transpose(pt, logits[:, ts(i, 128)], ident)
                    nc.vector.tensor_copy(lt[:, i], pt)
            b4r = b4bc[:]
            nc.vector.tensor_tensor(
                out=lt, in0=lt,
                in1=bass.AP(tensor=b4r.tensor, offset=b4r.offset,
                            ap=[b4r.ap[0], [0, BT], b4r.ap[1]]),
                op=ALU.add,
            )
            nc.scalar.activation(esb, lt, AF.Exp)
            nc.vector.tensor_reduce(
                out=lse, in_=esb, axis=mybir.AxisListType.X, op=ALU.add
            )
            nc.scalar.activation(lse, lse, AF.Ln)
            lser = lse[:]
            nc.vector.tensor_tensor(
                out=outf, in0=lt,
                in1=bass.AP(tensor=lser.tensor, offset=lser.offset,
                            ap=[lser.ap[0], lser.ap[1], [0, OUT_F]]),
                op=ALU.subtract,
            )
            nc.sync.dma_start(
                out=y_t.rearrange("(i p) o -> p i o", p=128), in_=outf
            )

    nc.finalize()
    return nc


def _host_prep(inputs):
    """Shard x, binarize/lay out weights, fold bn into sign biases."""
    import ml_dtypes

    f16 = np.float16
    f8 = ml_dtypes.float8_e4m3

    x = np.asarray(inputs["x"], np.float32)
    w1 = np.asarray(inputs["w1"], np.float32)
    w2 = np.asarray(inputs["w2"], np.float32)
    w3 = np.asarray(inputs["w3"], np.float32)
    w4 = np.asarray(inputs["w4"], np.float32)
    b1 = np.asarray(inputs["b1"], np.float32)
    b2 = np.asarray(inputs["b2"], np.float32)
    b3 = np.asarray(inputs["b3"], np.float32)
    b4 = np.asarray(inputs["b4"], np.float32)

    EPS = np.float64(1e-5)

    def gv(i):
        return (np.asarray(inputs[f"g{i}"], np.float32),
                np.asarray(inputs[f"be{i}"], np.float32),
                np.asarray(inputs[f"m{i}"], np.float32),
                np.asarray(inputs[f"v{i}"], np.float32))

    g1, be1, m1, v1 = gv(1)
    g2, be2, m2, v2 = gv(2)
    g3, be3, m3, v3 = gv(3)
    # sign(bn(h)) == sign(h + (b - m)) requires gamma > 0 and beta == 0
    assert np.all(g1 > 0) and np.all(be1 == 0), "unsupported bn1 params"
    assert np.all(g2 > 0) and np.all(be2 == 0), "unsupported bn2 params"

    bias1 = (b1 - m1).astype(np.float32)
    bias2 = (b2 - m2).astype(np.float32)
    r3 = 1.0 / np.sqrt(v3.astype(np.float64) + EPS)
    sc3 = (r3 * g3).astype(np.float32)
    sh3 = ((b3 - m3).astype(np.float64) * r3 * g3 + be3).astype(np.float32)

    def wlay(w, kc, dt):  # [out, in] -> [m, k, 128p(in), 128c(out)]
        st = np.sign(w).T.astype(np.float32)            # [in, out]
        kin = kc * 128
        if st.shape[0] < kin:
            st = np.pad(st, ((0, kin - st.shape[0]), (0, 0)))
        mo = st.shape[1] // 128
        return np.ascontiguousarray(
            st.reshape(kc, 128, mo, 128).transpose(2, 0, 1, 3)
        ).astype(dt)

    # s1: chunks 0-5 = rows 0-767; chunk 6 = packed tail (rows 768-783
    # replicated at partitions 0-15 and 32-47, matching the x tail chunk)
    s1f = np.sign(w1).T.astype(np.float32)              # [784, 1024]
    s1t = np.zeros((HC, K1C, 128, 128), np.float32)
    body = s1f[:768].reshape(6, 128, HC, 128)
    tail = s1f[768:784].reshape(16, HC, 128)
    for m in range(HC):
        s1t[m, :6] = body[:, :, m]
        s1t[m, 6, 0:16] = tail[:, m]
        s1t[m, 6, 32:48] = tail[:, m]
    s1t = s1t.astype(f16)
    s2t = wlay(w2, HC, f8)
    s3t = wlay(w3, HC, f8)
    w4t = np.ascontiguousarray(w4.T.astype(f16)).reshape(HC, 128, OUT_F)

    shared = dict(s1t=s1t, s2t=s2t, s3t=s3t, w4t=w4t,
                  bias1=bias1, bias2=bias2, sc3=sc3, sh3=sh3, b4=b4)
    in_maps = []
    for c in range(N_CORES):
        xs = x[c * BS:(c + 1) * BS]                     # [2048, 784]
        xt = xs.T                                       # [784, 2048]
        xhi = xt.astype(f16)
        xlo = (xt - xhi.astype(np.float32)).astype(f16)
        xc = np.zeros((XC, 128, BS), f16)
        xc[0:6] = xhi[:768].reshape(6, 128, BS)
        xc[6:12] = xlo[:768].reshape(6, 128, BS)
        xc[12, 0:16] = xhi[768:784]
        xc[12, 32:48] = xlo[768:784]
        m = dict(shared)
        m["xc"] = np.ascontiguousarray(xc)
        in_maps.append(m)
    return in_maps


def kernel(**inputs):
    global LAST_RESULT
    from concourse.bass_utils import run_bass_kernel_spmd

    if "nc" not in _PLAN:
        _PLAN["nc"] = _build_nc()
    nc = _PLAN["nc"]

    in_maps = _host_prep(inputs)
    br = run_bass_kernel_spmd(
        nc, in_maps, list(range(N_CORES)),
        tmpdir=os.environ.get("KERNEL_TMPDIR") or None,
    )
    LAST_RESULT = br
    out = np.concatenate([br.results[c]["y"] for c in range(N_CORES)], axis=0)
    return out.astype(np.float32)


# revision 9
# speedup vs baseline: 1.0288x; 1.0288x over previous
"""Binarized MLP (784 -> 1024 -> 1024 -> 1024 -> 10) on 8 TRN2 NeuronCores.

Data-parallel over the batch (16384 rows -> 2048 per core), weights replicated.

Math notes (these make the kernel both fast and numerically faithful):
  * Layers 1-2 outputs are only ever consumed through binarize(hardtanh(bn(h))).
    Since hardtanh preserves sign and bn here is (h - m) * rsqrt(v+eps) * g + be
    with g > 0, be == 0, the next-layer input is exactly sign(h + (b - m)).
    That is one ScalarE Sign activation with a per-partition bias, no bn needed.
  * fc2/fc3 multiply two +-1 operands -> exact in fp8(e4m3) with fp32 PSUM
    accumulation (integer partial sums, magnitude <= 1024). DoubleRow perf mode
    contracts two 128-row chunks per pass (2 fp8 weights per PE cell).
  * fc1 keeps x at full precision via an exact fp16 hi/lo split:
    x = hi + lo with hi = fp16(x), lo = fp16(x - hi); products with +-1 weights
    are exact, so accuracy ~ fp32 matmul, at 2 bf16-rate passes.
  * fc4 + log_softmax: logits computed feature-major [10, B], PE-transposed to
    [B, 10]; log_softmax without max-subtraction (logits are small; exp is safe).

Loop order: weights stationary per (m, k); all 4 batch column chunks stream
per weight load (amortizes LDWEIGHTS). 4 PSUM banks accumulate per m-tile,
8-slot pool double-buffers across m-tiles.
"""

import os
import numpy as np

N_CORES = 8
B_FULL = 16384
BS = B_FULL // N_CORES  # 2048 rows per core
IN_F = 784
K1C = 7                 # s1 weight chunks of 128 (784 padded to 896; chunk 6 = packed tail)
XC = 13                 # fc1 x chunks: 6 hi + 6 lo + 1 packed hi/lo tail
H = 1024
HC = 8                  # hidden chunks of 128
OUT_F = 10
NSPLIT = 4              # batch column chunks of 512
NB = BS // NSPLIT       # 512
BT = BS // 128          # 16 batch tiles of 128 for the output transpose

LAST_RESULT = None      # BassKernelResults of the most recent run (for test.py)

_PLAN = {}


def _build_nc():
    import concourse.bass as bass
    import concourse.mybir as mybir
    import concourse.tile as tile
    from concourse.tile import add_dep_helper
    from concourse import bacc
    from concourse.bass import ts
    from concourse.masks import make_identity

    f32 = mybir.dt.float32
    f16 = mybir.dt.float16
    f8 = mybir.dt.float8e4
    AF = mybir.ActivationFunctionType
    ALU = mybir.AluOpType
    DR = mybir.MatmulPerfMode.DoubleRow

    nc = bacc.Bacc(None)

    x_t = nc.dram_tensor("xc", [XC, 128, BS], f16, kind="ExternalInput")
    s1_t = nc.dram_tensor("s1t", [HC, K1C, 128, 128], f16, kind="ExternalInput")
    s2_t = nc.dram_tensor("s2t", [HC, HC, 128, 128], f8, kind="ExternalInput")
    s3_t = nc.dram_tensor("s3t", [HC, HC, 128, 128], f8, kind="ExternalInput")
    w4_t = nc.dram_tensor("w4t", [HC, 128, OUT_F], f16, kind="ExternalInput")
    b1_t = nc.dram_tensor("bias1", [H], f32, kind="ExternalInput")
    b2_t = nc.dram_tensor("bias2", [H], f32, kind="ExternalInput")
    sc3_t = nc.dram_tensor("sc3", [H], f32, kind="ExternalInput")
    sh3_t = nc.dram_tensor("sh3", [H], f32, kind="ExternalInput")
    b4_t = nc.dram_tensor("b4", [OUT_F], f32, kind="ExternalInput")
    y_t = nc.dram_tensor("y", [BS, OUT_F], f32, kind="ExternalOutput")

    with tile.TileContext(nc) as tc:
        with (
            tc.tile_pool(name="consts", bufs=1) as consts,
            tc.tile_pool(name="tmp", bufs=4) as tmp,
            tc.tile_pool(name="psum", bufs=8, space="PSUM") as psum,
        ):
            x_sb = consts.tile([128, XC, BS], f16, tag="xc")
            s1_sb = consts.tile([128, HC, K1C, 128], f16, tag="s1")
            s2_sb = consts.tile([128, HC, HC, 128], f8, tag="s2")
            s3_sb = consts.tile([128, HC, HC, 128], f8, tag="s3")
            w4_sb = consts.tile([128, HC, OUT_F], f16, tag="w4")
            b1v = consts.tile([128, HC], f32, tag="b1v")
            b2v = consts.tile([128, HC], f32, tag="b2v")
            sc3v = consts.tile([128, HC], f32, tag="sc3v")
            sh3v = consts.tile([128, HC], f32, tag="sh3v")
            b4bc = consts.tile([128, OUT_F], f32, tag="b4bc")
            ident = consts.tile([OUT_F, OUT_F], f32, tag="ident")
            act1 = consts.tile([128, HC, BS], f8, tag="act1")
            act2 = consts.tile([128, HC, BS], f8, tag="act2")
            act3 = consts.tile([128, HC, BS], f16, tag="act3")
            logits = consts.tile([OUT_F, BS], f32, tag="logits")
            lt = consts.tile([128, BT, OUT_F], f32, tag="lt")
            esb = consts.tile([128, BT, OUT_F], f32, tag="esb")
            lse = consts.tile([128, BT], f32, tag="lse")
            outf = consts.tile([128, BT, OUT_F], f32, tag="outf")

            # ---- input DMAs: x k0 + m=0 weights first, x chunks spread
            # over three issuing engines so enqueue parallelizes ----
            nc.sync.dma_start(out=x_sb[:, 0], in_=x_t[0])
            nc.scalar.dma_start(
                out=s1_sb[:, 0], in_=s1_t[0].rearrange("k p c -> p k c")
            )
            nc.gpsimd.dma_start(out=b1v, in_=b1_t[:].rearrange("(m p) -> p m", p=128))
            dma_engs = [nc.gpsimd, nc.scalar, nc.sync]
            for k in range(1, XC):
                dma_engs[k % 3].dma_start(out=x_sb[:, k], in_=x_t[k])
            for m in range(1, HC):
                nc.sync.dma_start(
                    out=s1_sb[:, m], in_=s1_t[m].rearrange("k p c -> p k c")
                )

            # ---- fc1: h1 = xT.T @ s1T (feature-major), sign -> act1 ----
            # x chunks: 0-5 = hi rows 0-767, 6-11 = lo rows 0-767,
            # 12 = packed tail (hi rows 768-783 @p0-15, lo @p32-47).
            for m in range(HC):
                pss = [psum.tile([128, NB], f32, tag="mm", name="ps") for _ in range(NSPLIT)]
                for k in range(XC):
                    wk = k if k < 6 else (k - 6 if k < 12 else 6)
                    for n in range(NSPLIT):
                        nc.tensor.matmul(
                            pss[n], s1_sb[:, m, wk], x_sb[:, k, ts(n, NB)],
                            start=(k == 0), stop=(k == XC - 1),
                        )
                for n in range(NSPLIT):
                    a = nc.scalar.activation(
                        act1[:, m, ts(n, NB)], pss[n], AF.Sign, bias=b1v[:, m:m + 1]
                    )
                    if m == 1 and n == NSPLIT - 1:
                        x_done_gate = a

            # later-layer weights: gated behind fc1 m=1 so their transfers
            # don't steal HBM bandwidth from the x load during the ramp
            for m in range(HC):
                d = nc.gpsimd.dma_start(
                    out=s2_sb[:, m], in_=s2_t[m].rearrange("k p c -> p k c")
                )
                add_dep_helper(d.ins, x_done_gate.ins, reason="defer s2 after x load")
            nc.sync.dma_start(out=b2v, in_=b2_t[:].rearrange("(m p) -> p m", p=128))
            for m in range(HC):
                d = nc.gpsimd.dma_start(
                    out=s3_sb[:, m], in_=s3_t[m].rearrange("k p c -> p k c")
                )
                add_dep_helper(d.ins, x_done_gate.ins, reason="defer s3 after x load")
            nc.sync.dma_start(out=sc3v, in_=sc3_t[:].rearrange("(m p) -> p m", p=128))
            nc.sync.dma_start(out=sh3v, in_=sh3_t[:].rearrange("(m p) -> p m", p=128))
            nc.sync.dma_start(out=w4_sb, in_=w4_t.rearrange("k p o -> p k o"))
            b4_ap = b4_t[:]
            nc.sync.dma_start(
                out=b4bc,
                in_=bass.AP(tensor=b4_ap.tensor, offset=b4_ap.offset,
                            ap=[[0, 128]] + list(b4_ap.ap)),
            )
            make_identity(nc, ident)

            # ---- fc2: binary x binary, fp8 DoubleRow, sign -> act2 ----
            for m in range(HC):
                pss = [psum.tile([128, NB], f32, tag="mm", name="ps") for _ in range(NSPLIT)]
                for kk in range(HC // 2):
                    ksl = slice(2 * kk, 2 * kk + 2)
                    for n in range(NSPLIT):
                        nc.tensor.matmul(
                            pss[n], s2_sb[:, m, ksl], act1[:, ksl, ts(n, NB)],
                            start=(kk == 0), stop=(kk == HC // 2 - 1),
                            perf_mode=DR,
                        )
                for n in range(NSPLIT):
                    nc.scalar.activation(
                        act2[:, m, ts(n, NB)], pss[n], AF.Sign, bias=b2v[:, m:m + 1]
                    )

            # ---- fc3: fp8 DoubleRow, bn affine + hardtanh -> act3 (DVE) ----
            for m in range(HC):
                pss = [psum.tile([128, NB], f32, tag="mm", name="ps") for _ in range(NSPLIT)]
                for kk in range(HC // 2):
                    ksl = slice(2 * kk, 2 * kk + 2)
                    for n in range(NSPLIT):
                        nc.tensor.matmul(
                            pss[n], s3_sb[:, m, ksl], act2[:, ksl, ts(n, NB)],
                            start=(kk == 0), stop=(kk == HC // 2 - 1),
                            perf_mode=DR,
                        )
                for n in range(NSPLIT):
                    t = tmp.tile([128, NB], f32, tag="t3")
                    nc.vector.tensor_scalar(
                        out=t, in0=pss[n],
                        scalar1=sc3v[:, m:m + 1], scalar2=sh3v[:, m:m + 1],
                        op0=ALU.mult, op1=ALU.add,
                    )
                    nc.vector.tensor_scalar(
                        out=act3[:, m, ts(n, NB)], in0=t,
                        scalar1=-1.0, scalar2=1.0,
                        op0=ALU.max, op1=ALU.min,
                    )

            # ---- fc4: logits[10, BS]; per-n copy + transpose pipelined ----
            for n in range(NSPLIT):
                ps4 = psum.tile([OUT_F, NB], f32, tag="mm", name="ps4")
                for k in range(HC):
                    nc.tensor.matmul(
                        ps4, w4_sb[:, k], act3[:, k, ts(n, NB)],
                        start=(k == 0), stop=(k == HC - 1),
                    )
                nc.scalar.copy(logits[:, ts(n, NB)], ps4)
                for i in range(4 * n, 4 * n + 4):
                    pt = psum.tile([128, OUT_F], f32, tag="mm")
                    nc.tensor.transpose(pt, logits[:, ts(i, 128)], ident)
                    nc.vector.tensor_copy(lt[:, i], pt)
            b4r = b4bc[:]
            nc.vector.tensor_tensor(
                out=lt, in0=lt,
                in1=bass.AP(tensor=b4r.tensor, offset=b4r.offset,
                            ap=[b4r.ap[0], [0, BT], b4r.ap[1]]),
                op=ALU.add,
            )
            nc.scalar.activation(esb, lt, AF.Exp)
            nc.vector.tensor_reduce(
                out=lse, in_=esb, axis=mybir.AxisListType.X, op=ALU.add
            )
            nc.scalar.activation(lse, lse, AF.Ln)
            lser = lse[:]
            nc.vector.tensor_tensor(
                out=outf, in0=lt,
                in1=bass.AP(tensor=lser.tensor, offset=lser.offset,
                            ap=[lser.ap[0], lser.ap[1], [0, OUT_F]]),
                op=ALU.subtract,
            )
            nc.sync.dma_start(
                out=y_t.rearrange("(i p) o -> p i o", p=128), in_=outf
            )

    nc.finalize()
    return nc


def _host_prep(inputs):
    """Shard x, binarize/lay out weights, fold bn into sign biases."""
    import ml_dtypes

    f16 = np.float16
    f8 = ml_dtypes.float8_e4m3

    x = np.asarray(inputs["x"], np.float32)
    w1 = np.asarray(inputs["w1"], np.float32)
    w2 = np.asarray(inputs["w2"], np.float32)
    w3 = np.asarray(inputs["w3"], np.float32)
    w4 = np.asarray(inputs["w4"], np.float32)
    b1 = np.asarray(inputs["b1"], np.float32)
    b2 = np.asarray(inputs["b2"], np.float32)
    b3 = np.asarray(inputs["b3"], np.float32)
    b4 = np.asarray(inputs["b4"], np.float32)

    EPS = np.float64(1e-5)

    def gv(i):
        return (np.asarray(inputs[f"g{i}"], np.float32),
                np.asarray(inputs[f"be{i}"], np.float32),
                np.asarray(inputs[f"m{i}"], np.float32),
                np.asarray(inputs[f"v{i}"], np.float32))

    g1, be1, m1, v1 = gv(1)
    g2, be2, m2, v2 = gv(2)
    g3, be3, m3, v3 = gv(3)
    # sign(bn(h)) == sign(h + (b - m)) requires gamma > 0 and beta == 0
    assert np.all(g1 > 0) and np.all(be1 == 0), "unsupported bn1 params"
    assert np.all(g2 > 0) and np.all(be2 == 0), "unsupported bn2 params"

    bias1 = (b1 - m1).astype(np.float32)
    bias2 = (b2 - m2).astype(np.float32)
    r3 = 1.0 / np.sqrt(v3.astype(np.float64) + EPS)
    sc3 = (r3 * g3).astype(np.float32)
    sh3 = ((b3 - m3).astype(np.float64) * r3 * g3 + be3).astype(np.float32)

    def wlay(w, kc, dt):  # [out, in] -> [m, k, 128p(in), 128c(out)]
        st = np.sign(w).T.astype(np.float32)            # [in, out]
        kin = kc * 128
        if st.shape[0] < kin:
            st = np.pad(st, ((0, kin - st.shape[0]), (0, 0)))
        mo = st.shape[1] // 128
        return np.ascontiguousarray(
            st.reshape(kc, 128, mo, 128).transpose(2, 0, 1, 3)
        ).astype(dt)

    # s1: chunks 0-5 = rows 0-767; chunk 6 = packed tail (rows 768-783
    # replicated at partitions 0-15 and 32-47, matching the x tail chunk)
    s1f = np.sign(w1).T.astype(np.float32)              # [784, 1024]
    s1t = np.zeros((HC, K1C, 128, 128), np.float32)
    body = s1f[:768].reshape(6, 128, HC, 128)
    tail = s1f[768:784].reshape(16, HC, 128)
    for m in range(HC):
        s1t[m, :6] = body[:, :, m]
        s1t[m, 6, 0:16] = tail[:, m]
        s1t[m, 6, 32:48] = tail[:, m]
    s1t = s1t.astype(f16)
    s2t = wlay(w2, HC, f8)
    s3t = wlay(w3, HC, f8)
    w4t = np.ascontiguousarray(w4.T.astype(f16)).reshape(HC, 128, OUT_F)

    shared = dict(s1t=s1t, s2t=s2t, s3t=s3t, w4t=w4t,
                  bias1=bias1, bias2=bias2, sc3=sc3, sh3=sh3, b4=b4)
    in_maps = []
    for c in range(N_CORES):
        xs = x[c * BS:(c + 1) * BS]                     # [2048, 784]
        xt = xs.T                                       # [784, 2048]
        xhi = xt.astype(f16)
        xlo = (xt - xhi.astype(np.float32)).astype(f16)
        xc = np.zeros((XC, 128, BS), f16)
        xc[0:6] = xhi[:768].reshape(6, 128, BS)
        xc[6:12] = xlo[:768].reshape(6, 128, BS)
        xc[12, 0:16] = xhi[768:784]
        xc[12, 32:48] = xlo[768:784]
        m = dict(shared)
        m["xc"] = np.ascontiguousarray(xc)
        in_maps.append(m)
    return in_maps


def kernel(**inputs):
    global LAST_RESULT
    from concourse.bass_utils import run_bass_kernel_spmd

    if "nc" not in _PLAN:
        _PLAN["nc"] = _build_nc()
    nc = _PLAN["nc"]

    in_maps = _host_prep(inputs)
    br = run_bass_kernel_spmd(
        nc, in_maps, list(range(N_CORES)),
        tmpdir=os.environ.get("KERNEL_TMPDIR") or None,
    )
    LAST_RESULT = br
    out = np.concatenate([br.results[c]["y"] for c in range(N_CORES)], axis=0)
    return out.astype(np.float32)


# revision 10
# speedup vs baseline: 1.0487x; 1.0194x over previous
"""Binarized MLP (784 -> 1024 -> 1024 -> 1024 -> 10) on 8 TRN2 NeuronCores.

Data-parallel over the batch (16384 rows -> 2048 per core), weights replicated.

Math notes (these make the kernel both fast and numerically faithful):
  * Layers 1-2 outputs are only ever consumed through binarize(hardtanh(bn(h))).
    Since hardtanh preserves sign and bn here is (h - m) * rsqrt(v+eps) * g + be
    with g > 0, be == 0, the next-layer input is exactly sign(h + (b - m)).
    That is one ScalarE Sign activation with a per-partition bias, no bn needed.
  * fc2/fc3 multiply two +-1 operands -> exact in fp8(e4m3) with fp32 PSUM
    accumulation (integer partial sums, magnitude <= 1024). DoubleRow perf mode
    contracts two 128-row chunks per pass (2 fp8 weights per PE cell).
  * fc1 keeps x at full precision via an exact fp16 hi/lo split:
    x = hi + lo with hi = fp16(x), lo = fp16(x - hi); products with +-1 weights
    are exact, so accuracy ~ fp32 matmul, at 2 bf16-rate passes.
  * fc4 + log_softmax: logits computed feature-major [10, B], PE-transposed to
    [B, 10]; log_softmax without max-subtraction (logits are small; exp is safe).

Loop order: weights stationary per (m, k); all 4 batch column chunks stream
per weight load (amortizes LDWEIGHTS). 4 PSUM banks accumulate per m-tile,
8-slot pool double-buffers across m-tiles.
"""

import os
import numpy as np

N_CORES = 8
B_FULL = 16384
BS = B_FULL // N_CORES  # 2048 rows per core
IN_F = 784
K1C = 7                 # s1 weight chunks of 128 (784 padded to 896; chunk 6 = packed tail)
XC = 13                 # fc1 x chunks: 6 hi + 6 lo + 1 packed hi/lo tail
H = 1024
HC = 8                  # hidden chunks of 128
OUT_F = 10
NSPLIT = 4              # batch column chunks of 512
NB = BS // NSPLIT       # 512
BT = BS // 128          # 16 batch tiles of 128 for the output transpose

LAST_RESULT = None      # BassKernelResults of the most recent run (for test.py)

_PLAN = {}


def _build_nc():
    import concourse.bass as bass
    import concourse.mybir as mybir
    import concourse.tile as tile
    from concourse.tile import add_dep_helper
    from concourse import bacc
    from concourse.bass import ts
    from concourse.masks import make_identity

    f32 = mybir.dt.float32
    f16 = mybir.dt.float16
    f8 = mybir.dt.float8e4
    AF = mybir.ActivationFunctionType
    ALU = mybir.AluOpType
    DR = mybir.MatmulPerfMode.DoubleRow

    nc = bacc.Bacc(None)

    x_t = nc.dram_tensor("xc", [XC, 128, BS], f16, kind="ExternalInput")
    s1_t = nc.dram_tensor("s1t", [HC, K1C, 128, 128], f16, kind="ExternalInput")
    s2_t = nc.dram_tensor("s2t", [HC, HC, 128, 128], f8, kind="ExternalInput")
    s3_t = nc.dram_tensor("s3t", [HC, HC, 128, 128], f8, kind="ExternalInput")
    w4_t = nc.dram_tensor("w4t", [HC, 128, OUT_F], f16, kind="ExternalInput")
    b1_t = nc.dram_tensor("bias1", [H], f32, kind="ExternalInput")
    b2_t = nc.dram_tensor("bias2", [H], f32, kind="ExternalInput")
    sc3_t = nc.dram_tensor("sc3", [H], f32, kind="ExternalInput")
    sh3_t = nc.dram_tensor("sh3", [H], f32, kind="ExternalInput")
    b4_t = nc.dram_tensor("b4", [OUT_F], f32, kind="ExternalInput")
    y_t = nc.dram_tensor("y", [BS, OUT_F], f32, kind="ExternalOutput")

    with tile.TileContext(nc) as tc:
        with (
            tc.tile_pool(name="consts", bufs=1) as consts,
            tc.tile_pool(name="tmp", bufs=4) as tmp,
            tc.tile_pool(name="psum", bufs=8, space="PSUM") as psum,
        ):
            x_sb = consts.tile([128, XC, BS], f16, tag="xc")
            s1_sb = consts.tile([128, HC, K1C, 128], f16, tag="s1")
            s2_sb = consts.tile([128, HC, HC, 128], f8, tag="s2")
            s3_sb = consts.tile([128, HC, HC, 128], f8, tag="s3")
            w4_sb = consts.tile([128, HC, OUT_F], f16, tag="w4")
            b1v = consts.tile([128, HC], f32, tag="b1v")
            b2v = consts.tile([128, HC], f32, tag="b2v")
            sc3v = consts.tile([128, HC], f32, tag="sc3v")
            sh3v = consts.tile([128, HC], f32, tag="sh3v")
            b4bc = consts.tile([128, OUT_F], f32, tag="b4bc")
            ident = consts.tile([OUT_F, OUT_F], f32, tag="ident")
            act1 = consts.tile([128, HC, BS], f8, tag="act1")
            act2 = consts.tile([128, HC, BS], f8, tag="act2")
            act3 = consts.tile([128, HC, BS], f16, tag="act3")
            logits = consts.tile([OUT_F, BS], f32, tag="logits")
            lt = consts.tile([128, BT, OUT_F], f32, tag="lt")
            esb = consts.tile([128, BT, OUT_F], f32, tag="esb")
            lse = consts.tile([128, BT], f32, tag="lse")
            outf = consts.tile([128, BT, OUT_F], f32, tag="outf")

            # ---- input DMAs: x k0 + m=0 weights first, x chunks spread
            # over three issuing engines so enqueue parallelizes ----
            nc.sync.dma_start(out=x_sb[:, 0], in_=x_t[0])
            nc.scalar.dma_start(
                out=s1_sb[:, 0], in_=s1_t[0].rearrange("k p c -> p k c")
            )
            nc.gpsimd.dma_start(out=b1v, in_=b1_t[:].rearrange("(m p) -> p m", p=128))
            # x on the two HWDGE rings, alternating: per-ring FIFO keeps
            # chunk completion in consumption order at full bandwidth
            dma_engs = [nc.sync, nc.scalar]
            for k in range(1, XC):
                dma_engs[k % 2].dma_start(out=x_sb[:, k], in_=x_t[k])
            for m in range(1, HC):
                nc.gpsimd.dma_start(
                    out=s1_sb[:, m], in_=s1_t[m].rearrange("k p c -> p k c")
                )

            # ---- fc1: h1 = xT.T @ s1T (feature-major), sign -> act1 ----
            # x chunks: 0-5 = hi rows 0-767, 6-11 = lo rows 0-767,
            # 12 = packed tail (hi rows 768-783 @p0-15, lo @p32-47).
            for m in range(HC):
                pss = [psum.tile([128, NB], f32, tag="mm", name="ps") for _ in range(NSPLIT)]
                for k in range(XC):
                    wk = k if k < 6 else (k - 6 if k < 12 else 6)
                    for n in range(NSPLIT):
                        nc.tensor.matmul(
                            pss[n], s1_sb[:, m, wk], x_sb[:, k, ts(n, NB)],
                            start=(k == 0), stop=(k == XC - 1),
                        )
                for n in range(NSPLIT):
                    a = nc.scalar.activation(
                        act1[:, m, ts(n, NB)], pss[n], AF.Sign, bias=b1v[:, m:m + 1]
                    )
                    if m == 1 and n == NSPLIT - 1:
                        x_done_gate = a

            # later-layer weights: gated behind fc1 m=1 so their transfers
            # don't steal HBM bandwidth from the x load during the ramp
            for m in range(HC):
                d = nc.gpsimd.dma_start(
                    out=s2_sb[:, m], in_=s2_t[m].rearrange("k p c -> p k c")
                )
                add_dep_helper(d.ins, x_done_gate.ins, reason="defer s2 after x load")
            nc.sync.dma_start(out=b2v, in_=b2_t[:].rearrange("(m p) -> p m", p=128))
            for m in range(HC):
                d = nc.gpsimd.dma_start(
                    out=s3_sb[:, m], in_=s3_t[m].rearrange("k p c -> p k c")
                )
                add_dep_helper(d.ins, x_done_gate.ins, reason="defer s3 after x load")
            nc.sync.dma_start(out=sc3v, in_=sc3_t[:].rearrange("(m p) -> p m", p=128))
            nc.sync.dma_start(out=sh3v, in_=sh3_t[:].rearrange("(m p) -> p m", p=128))
            nc.sync.dma_start(out=w4_sb, in_=w4_t.rearrange("k p o -> p k o"))
            b4_ap = b4_t[:]
            nc.sync.dma_start(
                out=b4bc,
                in_=bass.AP(tensor=b4_ap.tensor, offset=b4_ap.offset,
                            ap=[[0, 128]] + list(b4_ap.ap)),
            )
            make_identity(nc, ident)

            # ---- fc2: binary x binary, fp8 DoubleRow, sign -> act2 ----
            for m in range(HC):
                pss = [psum.tile([128, NB], f32, tag="mm", name="ps") for _ in range(NSPLIT)]
                for kk in range(HC // 2):
                    ksl = slice(2 * kk, 2 * kk + 2)
                    for n in range(NSPLIT):
                        nc.tensor.matmul(
                            pss[n], s2_sb[:, m, ksl], act1[:, ksl, ts(n, NB)],
                            start=(kk == 0), stop=(kk == HC // 2 - 1),
                            perf_mode=DR,
                        )
                for n in range(NSPLIT):
                    nc.scalar.activation(
                        act2[:, m, ts(n, NB)], pss[n], AF.Sign, bias=b2v[:, m:m + 1]
                    )

            # ---- fc3: fp8 DoubleRow, bn affine + hardtanh -> act3 (DVE) ----
            for m in range(HC):
                pss = [psum.tile([128, NB], f32, tag="mm", name="ps") for _ in range(NSPLIT)]
                for kk in range(HC // 2):
                    ksl = slice(2 * kk, 2 * kk + 2)
                    for n in range(NSPLIT):
                        nc.tensor.matmul(
                            pss[n], s3_sb[:, m, ksl], act2[:, ksl, ts(n, NB)],
                            start=(kk == 0), stop=(kk == HC // 2 - 1),
                            perf_mode=DR,
                        )
                for n in range(NSPLIT):
                    t = tmp.tile([128, NB], f32, tag="t3")
                    nc.scalar.activation(
                        t, pss[n], AF.Identity,
                        bias=sh3v[:, m:m + 1], scale=sc3v[:, m:m + 1],
                    )
                    nc.vector.tensor_scalar(
                        out=act3[:, m, ts(n, NB)], in0=t,
                        scalar1=-1.0, scalar2=1.0,
                        op0=ALU.max, op1=ALU.min,
                    )

            # ---- fc4: logits[10, BS]; per-n copy + transpose pipelined ----
            for n in range(NSPLIT):
                ps4 = psum.tile([OUT_F, NB], f32, tag="mm", name="ps4")
                for k in range(HC):
                    nc.tensor.matmul(
                        ps4, w4_sb[:, k], act3[:, k, ts(n, NB)],
                        start=(k == 0), stop=(k == HC - 1),
                    )
                nc.scalar.copy(logits[:, ts(n, NB)], ps4)
                for i in range(4 * n, 4 * n + 4):
                    pt = psum.tile([128, OUT_F], f32, tag="mm")
                    nc.tensor.transpose(pt, logits[:, ts(i, 128)], ident)
                    nc.vector.tensor_copy(lt[:, i], pt)
            b4r = b4bc[:]
            nc.vector.tensor_tensor(
                out=lt, in0=lt,
                in1=bass.AP(tensor=b4r.tensor, offset=b4r.offset,
                            ap=[b4r.ap[0], [0, BT], b4r.ap[1]]),
                op=ALU.add,
            )
            nc.scalar.activation(esb, lt, AF.Exp)
            nc.vector.tensor_reduce(
                out=lse, in_=esb, axis=mybir.AxisListType.X, op=ALU.add
            )
            nc.scalar.activation(lse, lse, AF.Ln)
            lser = lse[:]
            nc.vector.tensor_tensor(
                out=outf, in0=lt,
                in1=bass.AP(tensor=lser.tensor, offset=lser.offset,
                            ap=[lser.ap[0], lser.ap[1], [0, OUT_F]]),
                op=ALU.subtract,
            )
            nc.sync.dma_start(
                out=y_t.rearrange("(i p) o -> p i o", p=128), in_=outf
            )

    nc.finalize()
    return nc


def _host_prep(inputs):
    """Shard x, binarize/lay out weights, fold bn into sign biases."""
    import ml_dtypes

    f16 = np.float16
    f8 = ml_dtypes.float8_e4m3

    x = np.asarray(inputs["x"], np.float32)
    w1 = np.asarray(inputs["w1"], np.float32)
    w2 = np.asarray(inputs["w2"], np.float32)
    w3 = np.asarray(inputs["w3"], np.float32)
    w4 = np.asarray(inputs["w4"], np.float32)
    b1 = np.asarray(inputs["b1"], np.float32)
    b2 = np.asarray(inputs["b2"], np.float32)
    b3 = np.asarray(inputs["b3"], np.float32)
    b4 = np.asarray(inputs["b4"], np.float32)

    EPS = np.float64(1e-5)

    def gv(i):
        return (np.asarray(inputs[f"g{i}"], np.float32),
                np.asarray(inputs[f"be{i}"], np.float32),
                np.asarray(inputs[f"m{i}"], np.float32),
                np.asarray(inputs[f"v{i}"], np.float32))

    g1, be1, m1, v1 = gv(1)
    g2, be2, m2, v2 = gv(2)
    g3, be3, m3, v3 = gv(3)
    # sign(bn(h)) == sign(h + (b - m)) requires gamma > 0 and beta == 0
    assert np.all(g1 > 0) and np.all(be1 == 0), "unsupported bn1 params"
    assert np.all(g2 > 0) and np.all(be2 == 0), "unsupported bn2 params"

    bias1 = (b1 - m1).astype(np.float32)
    bias2 = (b2 - m2).astype(np.float32)
    r3 = 1.0 / np.sqrt(v3.astype(np.float64) + EPS)
    sc3 = (r3 * g3).astype(np.float32)
    sh3 = ((b3 - m3).astype(np.float64) * r3 * g3 + be3).astype(np.float32)

    def wlay(w, kc, dt):  # [out, in] -> [m, k, 128p(in), 128c(out)]
        st = np.sign(w).T.astype(np.float32)            # [in, out]
        kin = kc * 128
        if st.shape[0] < kin:
            st = np.pad(st, ((0, kin - st.shape[0]), (0, 0)))
        mo = st.shape[1] // 128
        return np.ascontiguousarray(
            st.reshape(kc, 128, mo, 128).transpose(2, 0, 1, 3)
        ).astype(dt)

    # s1: chunks 0-5 = rows 0-767; chunk 6 = packed tail (rows 768-783
    # replicated at partitions 0-15 and 32-47, matching the x tail chunk)
    s1f = np.sign(w1).T.astype(np.float32)              # [784, 1024]
    s1t = np.zeros((HC, K1C, 128, 128), np.float32)
    body = s1f[:768].reshape(6, 128, HC, 128)
    tail = s1f[768:784].reshape(16, HC, 128)
    for m in range(HC):
        s1t[m, :6] = body[:, :, m]
        s1t[m, 6, 0:16] = tail[:, m]
        s1t[m, 6, 32:48] = tail[:, m]
    s1t = s1t.astype(f16)
    s2t = wlay(w2, HC, f8)
    s3t = wlay(w3, HC, f8)
    w4t = np.ascontiguousarray(w4.T.astype(f16)).reshape(HC, 128, OUT_F)

    shared = dict(s1t=s1t, s2t=s2t, s3t=s3t, w4t=w4t,
                  bias1=bias1, bias2=bias2, sc3=sc3, sh3=sh3, b4=b4)
    in_maps = []
    for c in range(N_CORES):
        xs = x[c * BS:(c + 1) * BS]                     # [2048, 784]
        xt = xs.T                                       # [784, 2048]
        xhi = xt.astype(f16)
        xlo = (xt - xhi.astype(np.float32)).astype(f16)
        xc = np.zeros((XC, 128, BS), f16)
        xc[0:6] = xhi[:768].reshape(6, 128, BS)
        xc[6:12] = xlo[:768].reshape(6, 128, BS)
        xc[12, 0:16] = xhi[768:784]
        xc[12, 32:48] = xlo[768:784]
        m = dict(shared)
        m["xc"] = np.ascontiguousarray(xc)
        in_maps.append(m)
    return in_maps


def kernel(**inputs):
    global LAST_RESULT
    from concourse.bass_utils import run_bass_kernel_spmd

    if "nc" not in _PLAN:
        _PLAN["nc"] = _build_nc()
    nc = _PLAN["nc"]

    in_maps = _host_prep(inputs)
    br = run_bass_kernel_spmd(
        nc, in_maps, list(range(N_CORES)),
        tmpdir=os.environ.get("KERNEL_TMPDIR") or None,
    )
    LAST_RESULT = br
    out = np.concatenate([br.results[c]["y"] for c in range(N_CORES)], axis=0)
    return out.astype(np.float32)


# revision 11
# speedup vs baseline: 1.0808x; 1.0306x over previous
"""Binarized MLP (784 -> 1024 -> 1024 -> 1024 -> 10) on 8 TRN2 NeuronCores.

Data-parallel over the batch (16384 rows -> 2048 per core), weights replicated.

Math notes (these make the kernel both fast and numerically faithful):
  * Layers 1-2 outputs are only ever consumed through binarize(hardtanh(bn(h))).
    Since hardtanh preserves sign and bn here is (h - m) * rsqrt(v+eps) * g + be
    with g > 0, be == 0, the next-layer input is exactly sign(h + (b - m)).
    That is one ScalarE Sign activation with a per-partition bias, no bn needed.
  * fc2/fc3 multiply two +-1 operands -> exact in fp8(e4m3) with fp32 PSUM
    accumulation (integer partial sums, magnitude <= 1024). DoubleRow perf mode
    contracts two 128-row chunks per pass (2 fp8 weights per PE cell).
  * fc1 keeps x at full precision via an exact fp16 hi/lo split:
    x = hi + lo with hi = fp16(x), lo = fp16(x - hi); products with +-1 weights
    are exact, so accuracy ~ fp32 matmul, at 2 bf16-rate passes.
  * fc4 + log_softmax: logits computed feature-major [10, B], PE-transposed to
    [B, 10]; log_softmax without max-subtraction (logits are small; exp is safe).

Loop order: weights stationary per (m, k); all 4 batch column chunks stream
per weight load (amortizes LDWEIGHTS). 4 PSUM banks accumulate per m-tile,
8-slot pool double-buffers across m-tiles.
"""

import os
import numpy as np

N_CORES = 8
B_FULL = 16384
BS = B_FULL // N_CORES  # 2048 rows per core
IN_F = 784
K1C = 7                 # s1 weight chunks of 128 (784 padded to 896; chunk 6 = packed tail)
XC = 13                 # fc1 x chunks: 6 hi + 6 lo + 1 packed hi/lo tail
H = 1024
HC = 8                  # hidden chunks of 128
OUT_F = 10
NSPLIT = 4              # batch column chunks of 512
NB = BS // NSPLIT       # 512
BT = BS // 128          # 16 batch tiles of 128 for the output transpose

LAST_RESULT = None      # BassKernelResults of the most recent run (for test.py)

_PLAN = {}


def _build_nc():
    import concourse.bass as bass
    import concourse.mybir as mybir
    import concourse.tile as tile
    from concourse.tile import add_dep_helper
    from concourse import bacc
    from concourse.bass import ts
    from concourse.masks import make_identity

    f32 = mybir.dt.float32
    f16 = mybir.dt.float16
    f8 = mybir.dt.float8e4
    AF = mybir.ActivationFunctionType
    ALU = mybir.AluOpType
    DR = mybir.MatmulPerfMode.DoubleRow

    nc = bacc.Bacc(None)

    x_t = nc.dram_tensor("xc", [XC, 128, BS], f16, kind="ExternalInput")
    s1_t = nc.dram_tensor("s1t", [HC, K1C, 128, 128], f16, kind="ExternalInput")
    s2_t = nc.dram_tensor("s2t", [HC, HC, 128, 128], f8, kind="ExternalInput")
    s3_t = nc.dram_tensor("s3t", [HC, HC, 128, 128], f8, kind="ExternalInput")
    w4_t = nc.dram_tensor("w4t", [HC, 128, OUT_F], f16, kind="ExternalInput")
    b1_t = nc.dram_tensor("bias1", [H], f32, kind="ExternalInput")
    b2_t = nc.dram_tensor("bias2", [H], f32, kind="ExternalInput")
    sc3_t = nc.dram_tensor("sc3", [H], f32, kind="ExternalInput")
    sh3_t = nc.dram_tensor("sh3", [H], f32, kind="ExternalInput")
    b4_t = nc.dram_tensor("b4", [OUT_F], f32, kind="ExternalInput")
    y_t = nc.dram_tensor("y", [BS, OUT_F], f32, kind="ExternalOutput")

    with tile.TileContext(nc) as tc:
        with (
            tc.tile_pool(name="consts", bufs=1) as consts,
            tc.tile_pool(name="tmp", bufs=4) as tmp,
            tc.tile_pool(name="psum", bufs=8, space="PSUM") as psum,
        ):
            x_sb = consts.tile([128, XC, BS], f16, tag="xc")
            s1_sb = consts.tile([128, HC, K1C, 128], f16, tag="s1")
            s2_sb = consts.tile([128, HC, HC, 128], f8, tag="s2")
            s3_sb = consts.tile([128, HC, HC, 128], f8, tag="s3")
            w4_sb = consts.tile([128, HC, OUT_F], f16, tag="w4")
            b1v = consts.tile([128, HC], f32, tag="b1v")
            b2v = consts.tile([128, HC], f32, tag="b2v")
            sc3v = consts.tile([128, HC], f32, tag="sc3v")
            sh3v = consts.tile([128, HC], f32, tag="sh3v")
            b4bc = consts.tile([128, OUT_F], f32, tag="b4bc")
            ident = consts.tile([OUT_F, OUT_F], f32, tag="ident")
            act1 = consts.tile([128, HC, BS], f8, tag="act1")
            act2 = consts.tile([128, HC, BS], f8, tag="act2")
            act3 = consts.tile([128, HC, BS], f16, tag="act3")
            logits = consts.tile([OUT_F, BS], f32, tag="logits")
            lt = consts.tile([128, BT, OUT_F], f32, tag="lt")
            esb = consts.tile([128, BT, OUT_F], f32, tag="esb")
            lse = consts.tile([128, BT], f32, tag="lse")
            outf = consts.tile([128, BT, OUT_F], f32, tag="outf")

            # ---- input DMAs. First-needed pieces are split fine so the
            # PE can start as soon as possible; x rides the two HWDGE
            # rings, alternating (per-ring FIFO keeps chunk completion in
            # consumption order at full bandwidth); s1 m-tiles staggered.
            nc.gpsimd.dma_start(
                out=s1_sb[:, 0, 0:2], in_=s1_t[0, 0:2].rearrange("k p c -> p k c")
            )
            for n in range(NSPLIT):
                nc.sync.dma_start(out=x_sb[:, 0, ts(n, NB)], in_=x_t[0, :, ts(n, NB)])
            nc.scalar.dma_start(
                out=s1_sb[:, 0, 2:K1C], in_=s1_t[0, 2:K1C].rearrange("k p c -> p k c")
            )
            nc.gpsimd.dma_start(out=b1v, in_=b1_t[:].rearrange("(m p) -> p m", p=128))
            dma_engs = [nc.sync, nc.scalar]
            for k in range(1, XC):
                dma_engs[k % 2].dma_start(out=x_sb[:, k], in_=x_t[k])
            s1_dmas = {}
            for m in range(1, HC):
                s1_dmas[m] = nc.gpsimd.dma_start(
                    out=s1_sb[:, m], in_=s1_t[m].rearrange("k p c -> p k c")
                )

            # ---- fc1: h1 = xT.T @ s1T (feature-major), sign -> act1 ----
            # x chunks: 0-5 = hi rows 0-767, 6-11 = lo rows 0-767,
            # 12 = packed tail (hi rows 768-783 @p0-15, lo @p32-47).
            for m in range(HC):
                pss = [psum.tile([128, NB], f32, tag="mm", name="ps") for _ in range(NSPLIT)]
                for k in range(XC):
                    wk = k if k < 6 else (k - 6 if k < 12 else 6)
                    for n in range(NSPLIT):
                        nc.tensor.matmul(
                            pss[n], s1_sb[:, m, wk], x_sb[:, k, ts(n, NB)],
                            start=(k == 0), stop=(k == XC - 1),
                        )
                for n in range(NSPLIT):
                    a = nc.scalar.activation(
                        act1[:, m, ts(n, NB)], pss[n], AF.Sign, bias=b1v[:, m:m + 1]
                    )
                    if m == 1 and n == NSPLIT - 1:
                        x_done_gate = a
                    # stagger s1 weight loads two m-tiles ahead of use
                    if n == 0 and m + 3 in s1_dmas:
                        add_dep_helper(s1_dmas[m + 3].ins, a.ins,
                                       reason="stagger s1 loads")

            # later-layer weights: gated behind fc1 m=1 so their transfers
            # don't steal HBM bandwidth from the x load during the ramp
            for m in range(HC):
                d = nc.gpsimd.dma_start(
                    out=s2_sb[:, m], in_=s2_t[m].rearrange("k p c -> p k c")
                )
                add_dep_helper(d.ins, x_done_gate.ins, reason="defer s2 after x load")
            nc.sync.dma_start(out=b2v, in_=b2_t[:].rearrange("(m p) -> p m", p=128))
            for m in range(HC):
                d = nc.gpsimd.dma_start(
                    out=s3_sb[:, m], in_=s3_t[m].rearrange("k p c -> p k c")
                )
                add_dep_helper(d.ins, x_done_gate.ins, reason="defer s3 after x load")
            nc.sync.dma_start(out=sc3v, in_=sc3_t[:].rearrange("(m p) -> p m", p=128))
            nc.sync.dma_start(out=sh3v, in_=sh3_t[:].rearrange("(m p) -> p m", p=128))
            nc.sync.dma_start(out=w4_sb, in_=w4_t.rearrange("k p o -> p k o"))
            b4_ap = b4_t[:]
            nc.sync.dma_start(
                out=b4bc,
                in_=bass.AP(tensor=b4_ap.tensor, offset=b4_ap.offset,
                            ap=[[0, 128]] + list(b4_ap.ap)),
            )
            make_identity(nc, ident)

            # ---- fc2: binary x binary, fp8 DoubleRow, sign -> act2 ----
            for m in range(HC):
                pss = [psum.tile([128, NB], f32, tag="mm", name="ps") for _ in range(NSPLIT)]
                for kk in range(HC // 2):
                    ksl = slice(2 * kk, 2 * kk + 2)
                    for n in range(NSPLIT):
                        nc.tensor.matmul(
                            pss[n], s2_sb[:, m, ksl], act1[:, ksl, ts(n, NB)],
                            start=(kk == 0), stop=(kk == HC // 2 - 1),
                            perf_mode=DR,
                        )
                for n in range(NSPLIT):
                    nc.scalar.activation(
                        act2[:, m, ts(n, NB)], pss[n], AF.Sign, bias=b2v[:, m:m + 1]
                    )

            # ---- fc3: fp8 DoubleRow, bn affine + hardtanh -> act3 (DVE) ----
            for m in range(HC):
                pss = [psum.tile([128, NB], f32, tag="mm", name="ps") for _ in range(NSPLIT)]
                for kk in range(HC // 2):
                    ksl = slice(2 * kk, 2 * kk + 2)
                    for n in range(NSPLIT):
                        nc.tensor.matmul(
                            pss[n], s3_sb[:, m, ksl], act2[:, ksl, ts(n, NB)],
                            start=(kk == 0), stop=(kk == HC // 2 - 1),
                            perf_mode=DR,
                        )
                for n in range(NSPLIT):
                    t = tmp.tile([128, NB], f32, tag="t3")
                    nc.scalar.activation(
                        t, pss[n], AF.Identity,
                        bias=sh3v[:, m:m + 1], scale=sc3v[:, m:m + 1],
                    )
                    nc.vector.tensor_scalar(
                        out=act3[:, m, ts(n, NB)], in0=t,
                        scalar1=-1.0, scalar2=1.0,
                        op0=ALU.max, op1=ALU.min,
                    )

            # ---- fc4: logits[10, BS]; per-n copy + transpose pipelined ----
            for n in range(NSPLIT):
                ps4 = psum.tile([OUT_F, NB], f32, tag="mm", name="ps4")
                for k in range(HC):
                    nc.tensor.matmul(
                        ps4, w4_sb[:, k], act3[:, k, ts(n, NB)],
                        start=(k == 0), stop=(k == HC - 1),
                    )
                nc.scalar.copy(logits[:, ts(n, NB)], ps4)
                for i in range(4 * n, 4 * n + 4):
                    pt = psum.tile([128, OUT_F], f32, tag="mm")
                    nc.tensor.transpose(pt, logits[:, ts(i, 128)], ident)
                    nc.vector.tensor_copy(lt[:, i], pt)
            b4r = b4bc[:]
            nc.vector.tensor_tensor(
                out=lt, in0=lt,
                in1=bass.AP(tensor=b4r.tensor, offset=b4r.offset,
                            ap=[b4r.ap[0], [0, BT], b4r.ap[1]]),
                op=ALU.add,
            )
            nc.scalar.activation(esb, lt, AF.Exp)
            nc.vector.tensor_reduce(
                out=lse, in_=esb, axis=mybir.AxisListType.X, op=ALU.add
            )
            nc.scalar.activation(lse, lse, AF.Ln)
            lser = lse[:]
            nc.vector.tensor_tensor(
                out=outf, in0=lt,
                in1=bass.AP(tensor=lser.tensor, offset=lser.offset,
                            ap=[lser.ap[0], lser.ap[1], [0, OUT_F]]),
                op=ALU.subtract,
            )
            nc.sync.dma_start(
                out=y_t.rearrange("(i p) o -> p i o", p=128), in_=outf
            )

    nc.finalize()
    return nc


def _host_prep(inputs):
    """Shard x, binarize/lay out weights, fold bn into sign biases."""
    import ml_dtypes

    f16 = np.float16
    f8 = ml_dtypes.float8_e4m3

    x = np.asarray(inputs["x"], np.float32)
    w1 = np.asarray(inputs["w1"], np.float32)
    w2 = np.asarray(inputs["w2"], np.float32)
    w3 = np.asarray(inputs["w3"], np.float32)
    w4 = np.asarray(inputs["w4"], np.float32)
    b1 = np.asarray(inputs["b1"], np.float32)
    b2 = np.asarray(inputs["b2"], np.float32)
    b3 = np.asarray(inputs["b3"], np.float32)
    b4 = np.asarray(inputs["b4"], np.float32)

    EPS = np.float64(1e-5)

    def gv(i):
        return (np.asarray(inputs[f"g{i}"], np.float32),
                np.asarray(inputs[f"be{i}"], np.float32),
                np.asarray(inputs[f"m{i}"], np.float32),
                np.asarray(inputs[f"v{i}"], np.float32))

    g1, be1, m1, v1 = gv(1)
    g2, be2, m2, v2 = gv(2)
    g3, be3, m3, v3 = gv(3)
    # sign(bn(h)) == sign(h + (b - m)) requires gamma > 0 and beta == 0
    assert np.all(g1 > 0) and np.all(be1 == 0), "unsupported bn1 params"
    assert np.all(g2 > 0) and np.all(be2 == 0), "unsupported bn2 params"

    bias1 = (b1 - m1).astype(np.float32)
    bias2 = (b2 - m2).astype(np.float32)
    r3 = 1.0 / np.sqrt(v3.astype(np.float64) + EPS)
    sc3 = (r3 * g3).astype(np.float32)
    sh3 = ((b3 - m3).astype(np.float64) * r3 * g3 + be3).astype(np.float32)

    def wlay(w, kc, dt):  # [out, in] -> [m, k, 128p(in), 128c(out)]
        st = np.sign(w).T.astype(np.float32)            # [in, out]
        kin = kc * 128
        if st.shape[0] < kin:
            st = np.pad(st, ((0, kin - st.shape[0]), (0, 0)))
        mo = st.shape[1] // 128
        return np.ascontiguousarray(
            st.reshape(kc, 128, mo, 128).transpose(2, 0, 1, 3)
        ).astype(dt)

    # s1: chunks 0-5 = rows 0-767; chunk 6 = packed tail (rows 768-783
    # replicated at partitions 0-15 and 32-47, matching the x tail chunk)
    s1f = np.sign(w1).T.astype(np.float32)              # [784, 1024]
    s1t = np.zeros((HC, K1C, 128, 128), np.float32)
    body = s1f[:768].reshape(6, 128, HC, 128)
    tail = s1f[768:784].reshape(16, HC, 128)
    for m in range(HC):
        s1t[m, :6] = body[:, :, m]
        s1t[m, 6, 0:16] = tail[:, m]
        s1t[m, 6, 32:48] = tail[:, m]
    s1t = s1t.astype(f16)
    s2t = wlay(w2, HC, f8)
    s3t = wlay(w3, HC, f8)
    w4t = np.ascontiguousarray(w4.T.astype(f16)).reshape(HC, 128, OUT_F)

    shared = dict(s1t=s1t, s2t=s2t, s3t=s3t, w4t=w4t,
                  bias1=bias1, bias2=bias2, sc3=sc3, sh3=sh3, b4=b4)
    in_maps = []
    for c in range(N_CORES):
        xs = x[c * BS:(c + 1) * BS]                     # [2048, 784]
        xt = xs.T                                       # [784, 2048]
        xhi = xt.astype(f16)
        xlo = (xt - xhi.astype(np.float32)).astype(f16)
        xc = np.zeros((XC, 128, BS), f16)
        xc[0:6] = xhi[:768].reshape(6, 128, BS)
        xc[6:12] = xlo[:768].reshape(6, 128, BS)
        xc[12, 0:16] = xhi[768:784]
        xc[12, 32:48] = xlo[768:784]
        m = dict(shared)
        m["xc"] = np.ascontiguousarray(xc)
        in_maps.append(m)
    return in_maps


def kernel(**inputs):
    global LAST_RESULT
    from concourse.bass_utils import run_bass_kernel_spmd

    if "nc" not in _PLAN:
        _PLAN["nc"] = _build_nc()
    nc = _PLAN["nc"]

    in_maps = _host_prep(inputs)
    br = run_bass_kernel_spmd(
        nc, in_maps, list(range(N_CORES)),
        tmpdir=os.environ.get("KERNEL_TMPDIR") or None,
    )
    LAST_RESULT = br
    out = np.concatenate([br.results[c]["y"] for c in range(N_CORES)], axis=0)
    return out.astype(np.float32)


# revision 13
# speedup vs baseline: 1.0958x; 1.0138x over previous
"""Binarized MLP (784 -> 1024 -> 1024 -> 1024 -> 10) on 8 TRN2 NeuronCores.

Data-parallel over the batch (16384 rows -> 2048 per core), weights replicated.

Math notes (these make the kernel both fast and numerically faithful):
  * Layers 1-2 outputs are only ever consumed through binarize(hardtanh(bn(h))).
    Since hardtanh preserves sign and bn here is (h - m) * rsqrt(v+eps) * g + be
    with g > 0, be == 0, the next-layer input is exactly sign(h + (b - m)).
    That is one ScalarE Sign activation with a per-partition bias, no bn needed.
  * fc2/fc3 multiply two +-1 operands -> exact in fp8(e4m3) with fp32 PSUM
    accumulation (integer partial sums, magnitude <= 1024). DoubleRow perf mode
    contracts two 128-row chunks per pass (2 fp8 weights per PE cell).
  * fc1 keeps x at full precision via an exact fp16 hi/lo split:
    x = hi + lo with hi = fp16(x), lo = fp16(x - hi); products with +-1 weights
    are exact, so accuracy ~ fp32 matmul, at 2 bf16-rate passes.
  * fc4 + log_softmax: logits computed feature-major [10, B], PE-transposed to
    [B, 10]; log_softmax without max-subtraction (logits are small; exp is safe).

Loop order: weights stationary per (m, k); all 4 batch column chunks stream
per weight load (amortizes LDWEIGHTS). 4 PSUM banks accumulate per m-tile,
8-slot pool double-buffers across m-tiles.
"""

import os
import numpy as np

N_CORES = 8
B_FULL = 16384
BS = B_FULL // N_CORES  # 2048 rows per core
IN_F = 784
K1C = 7                 # s1 weight chunks of 128 (784 padded to 896; chunk 6 = packed tail)
XC = 13                 # fc1 x chunks: 6 hi + 6 lo + 1 packed hi/lo tail
H = 1024
HC = 8                  # hidden chunks of 128
OUT_F = 10
NSPLIT = 4              # batch column chunks of 512
NB = BS // NSPLIT       # 512
BT = BS // 128          # 16 batch tiles of 128 for the output transpose

LAST_RESULT = None      # BassKernelResults of the most recent run (for test.py)

_PLAN = {}


def _build_nc():
    import concourse.bass as bass
    import concourse.mybir as mybir
    import concourse.tile as tile
    from concourse.tile import add_dep_helper
    from concourse import bacc
    from concourse.bass import ts
    from concourse.masks import make_identity

    f32 = mybir.dt.float32
    f16 = mybir.dt.float16
    f8 = mybir.dt.float8e4
    AF = mybir.ActivationFunctionType
    ALU = mybir.AluOpType
    DR = mybir.MatmulPerfMode.DoubleRow

    nc = bacc.Bacc(None)

    x_t = nc.dram_tensor("xc", [XC, 128, BS], f16, kind="ExternalInput")
    s1_t = nc.dram_tensor("s1t", [HC, K1C, 128, 128], f16, kind="ExternalInput")
    s2_t = nc.dram_tensor("s2t", [HC, HC, 128, 128], f8, kind="ExternalInput")
    s3_t = nc.dram_tensor("s3t", [HC, HC, 128, 128], f8, kind="ExternalInput")
    w4_t = nc.dram_tensor("w4t", [HC, 128, OUT_F], f16, kind="ExternalInput")
    b1_t = nc.dram_tensor("bias1", [H], f32, kind="ExternalInput")
    b2_t = nc.dram_tensor("bias2", [H], f32, kind="ExternalInput")
    sc3_t = nc.dram_tensor("sc3", [H], f32, kind="ExternalInput")
    sh3_t = nc.dram_tensor("sh3", [H], f32, kind="ExternalInput")
    b4_t = nc.dram_tensor("b4", [OUT_F], f32, kind="ExternalInput")
    y_t = nc.dram_tensor("y", [BS, OUT_F], f32, kind="ExternalOutput")

    with tile.TileContext(nc) as tc:
        with (
            tc.tile_pool(name="consts", bufs=1) as consts,
            tc.tile_pool(name="tmp", bufs=4) as tmp,
            tc.tile_pool(name="psum", bufs=8, space="PSUM") as psum,
        ):
            x_sb = consts.tile([128, XC, BS], f16, tag="xc")
            s1_sb = consts.tile([128, HC, K1C, 128], f16, tag="s1")
            s2_sb = consts.tile([128, HC, HC, 128], f8, tag="s2")
            s3_sb = consts.tile([128, HC, HC, 128], f8, tag="s3")
            w4_sb = consts.tile([128, HC, OUT_F], f16, tag="w4")
            b1v = consts.tile([128, HC], f32, tag="b1v")
            b2v = consts.tile([128, HC], f32, tag="b2v")
            sc3v = consts.tile([128, HC], f32, tag="sc3v")
            sh3v = consts.tile([128, HC], f32, tag="sh3v")
            b4bc = consts.tile([128, OUT_F], f32, tag="b4bc")
            act1 = consts.tile([128, HC, BS], f8, tag="act1")
            act2 = consts.tile([128, HC, BS], f8, tag="act2")
            act3 = consts.tile([128, HC, BS], f16, tag="act3")
            NBLK = BS // 32  # 64 batch blocks of 32 for the DVE transpose
            logits = consts.tile([32, BS], f32, tag="logits")
            ltr = consts.tile([32, BS], f32, tag="ltr")
            es2 = consts.tile([32, NBLK, OUT_F], f32, tag="es2")
            lse2 = consts.tile([32, NBLK], f32, tag="lse2")
            outf2 = consts.tile([32, NBLK, OUT_F], f32, tag="outf2")

            # ---- input DMAs. First-needed pieces are split fine so the
            # PE can start as soon as possible; x rides the two HWDGE
            # rings, alternating (per-ring FIFO keeps chunk completion in
            # consumption order at full bandwidth); s1 m-tiles staggered.
            nc.gpsimd.dma_start(
                out=s1_sb[:, 0, 0:2], in_=s1_t[0, 0:2].rearrange("k p c -> p k c")
            )
            for n in range(NSPLIT):
                nc.sync.dma_start(out=x_sb[:, 0, ts(n, NB)], in_=x_t[0, :, ts(n, NB)])
            nc.scalar.dma_start(
                out=s1_sb[:, 0, 2:K1C], in_=s1_t[0, 2:K1C].rearrange("k p c -> p k c")
            )
            nc.gpsimd.dma_start(out=b1v, in_=b1_t[:].rearrange("(m p) -> p m", p=128))
            dma_engs = [nc.sync, nc.scalar]
            for k in range(1, XC):
                dma_engs[k % 2].dma_start(out=x_sb[:, k], in_=x_t[k])
            s1_dmas = {}
            for m in range(1, HC):
                s1_dmas[m] = nc.gpsimd.dma_start(
                    out=s1_sb[:, m], in_=s1_t[m].rearrange("k p c -> p k c")
                )

            # ---- fc1: h1 = xT.T @ s1T (feature-major), sign -> act1 ----
            # x chunks: 0-5 = hi rows 0-767, 6-11 = lo rows 0-767,
            # 12 = packed tail (hi rows 768-783 @p0-15, lo @p32-47).
            for m in range(HC):
                pss = [psum.tile([128, NB], f32, tag="mm", name="ps") for _ in range(NSPLIT)]
                for k in range(XC):
                    wk = k if k < 6 else (k - 6 if k < 12 else 6)
                    for n in range(NSPLIT):
                        nc.tensor.matmul(
                            pss[n], s1_sb[:, m, wk], x_sb[:, k, ts(n, NB)],
                            start=(k == 0), stop=(k == XC - 1),
                        )
                for n in range(NSPLIT):
                    a = nc.scalar.activation(
                        act1[:, m, ts(n, NB)], pss[n], AF.Sign, bias=b1v[:, m:m + 1]
                    )
                    if m == 1 and n == NSPLIT - 1:
                        x_done_gate = a
                    # stagger s1 weight loads two m-tiles ahead of use
                    if n == 0 and m + 3 in s1_dmas:
                        add_dep_helper(s1_dmas[m + 3].ins, a.ins,
                                       reason="stagger s1 loads")

            # later-layer weights: gated behind fc1 m=1 so their transfers
            # don't steal HBM bandwidth from the x load during the ramp
            for m in range(HC):
                d = nc.gpsimd.dma_start(
                    out=s2_sb[:, m], in_=s2_t[m].rearrange("k p c -> p k c")
                )
                add_dep_helper(d.ins, x_done_gate.ins, reason="defer s2 after x load")
            nc.sync.dma_start(out=b2v, in_=b2_t[:].rearrange("(m p) -> p m", p=128))
            for m in range(HC):
                d = nc.gpsimd.dma_start(
                    out=s3_sb[:, m], in_=s3_t[m].rearrange("k p c -> p k c")
                )
                add_dep_helper(d.ins, x_done_gate.ins, reason="defer s3 after x load")
            nc.sync.dma_start(out=sc3v, in_=sc3_t[:].rearrange("(m p) -> p m", p=128))
            nc.sync.dma_start(out=sh3v, in_=sh3_t[:].rearrange("(m p) -> p m", p=128))
            nc.sync.dma_start(out=w4_sb, in_=w4_t.rearrange("k p o -> p k o"))
            b4_ap = b4_t[:]
            nc.sync.dma_start(
                out=b4bc,
                in_=bass.AP(tensor=b4_ap.tensor, offset=b4_ap.offset,
                            ap=[[0, 128]] + list(b4_ap.ap)),
            )
            nc.vector.memset(logits, 0.0)

            # ---- fc2: binary x binary, fp8 DoubleRow, sign -> act2 ----
            for m in range(HC):
                pss = [psum.tile([128, NB], f32, tag="mm", name="ps") for _ in range(NSPLIT)]
                for kk in range(HC // 2):
                    ksl = slice(2 * kk, 2 * kk + 2)
                    for n in range(NSPLIT):
                        nc.tensor.matmul(
                            pss[n], s2_sb[:, m, ksl], act1[:, ksl, ts(n, NB)],
                            start=(kk == 0), stop=(kk == HC // 2 - 1),
                            perf_mode=DR,
                        )
                for n in range(NSPLIT):
                    nc.scalar.activation(
                        act2[:, m, ts(n, NB)], pss[n], AF.Sign, bias=b2v[:, m:m + 1]
                    )

            # ---- fc3: fp8 DoubleRow, bn affine + hardtanh -> act3 (DVE) ----
            for m in range(HC):
                pss = [psum.tile([128, NB], f32, tag="mm", name="ps") for _ in range(NSPLIT)]
                for kk in range(HC // 2):
                    ksl = slice(2 * kk, 2 * kk + 2)
                    for n in range(NSPLIT):
                        nc.tensor.matmul(
                            pss[n], s3_sb[:, m, ksl], act2[:, ksl, ts(n, NB)],
                            start=(kk == 0), stop=(kk == HC // 2 - 1),
                            perf_mode=DR,
                        )
                for n in range(NSPLIT):
                    t = tmp.tile([128, NB], f32, tag="t3")
                    nc.scalar.activation(
                        t, pss[n], AF.Identity,
                        bias=sh3v[:, m:m + 1], scale=sc3v[:, m:m + 1],
                    )
                    nc.vector.tensor_scalar(
                        out=act3[:, m, ts(n, NB)], in0=t,
                        scalar1=-1.0, scalar2=1.0,
                        op0=ALU.max, op1=ALU.min,
                    )

            # ---- fc4: logits[10, BS]; DVE 32x32 block transpose per n ----
            # ltr[p, 32j+q] = logit class q of batch row 32j+p  (q < 10)
            for n in range(NSPLIT):
                ps4 = psum.tile([OUT_F, NB], f32, tag="mm", name="ps4")
                for k in range(HC):
                    nc.tensor.matmul(
                        ps4, w4_sb[:, k], act3[:, k, ts(n, NB)],
                        start=(k == 0), stop=(k == HC - 1),
                    )
                nc.scalar.copy(logits[0:OUT_F, ts(n, NB)], ps4)
                nc.vector.transpose(ltr[:, ts(n, NB)], logits[:, ts(n, NB)])
            ltv = bass.AP(tensor=ltr[:].tensor, offset=ltr[:].offset,
                          ap=[ltr[:].ap[0], [32, NBLK], [1, OUT_F]])
            b4r = b4bc[:]
            nc.vector.tensor_tensor(
                out=ltv, in0=ltv,
                in1=bass.AP(tensor=b4r.tensor, offset=b4r.offset,
                            ap=[[b4r.ap[0][0], 32], [0, NBLK], b4r.ap[1]]),
                op=ALU.add,
            )
            nc.scalar.activation(es2, ltv, AF.Exp)
            nc.vector.tensor_reduce(
                out=lse2, in_=es2, axis=mybir.AxisListType.X, op=ALU.add
            )
            nc.scalar.activation(lse2, lse2, AF.Ln)
            lser = lse2[:]
            nc.vector.tensor_tensor(
                out=outf2, in0=ltv,
                in1=bass.AP(tensor=lser.tensor, offset=lser.offset,
                            ap=[lser.ap[0], lser.ap[1], [0, OUT_F]]),
                op=ALU.subtract,
            )
            nc.sync.dma_start(
                out=y_t.rearrange("(j p) o -> p j o", p=32), in_=outf2
            )

    nc.finalize()
    return nc


def _host_prep(inputs):
    """Shard x, binarize/lay out weights, fold bn into sign biases."""
    import ml_dtypes

    f16 = np.float16
    f8 = ml_dtypes.float8_e4m3

    x = np.asarray(inputs["x"], np.float32)
    w1 = np.asarray(inputs["w1"], np.float32)
    w2 = np.asarray(inputs["w2"], np.float32)
    w3 = np.asarray(inputs["w3"], np.float32)
    w4 = np.asarray(inputs["w4"], np.float32)
    b1 = np.asarray(inputs["b1"], np.float32)
    b2 = np.asarray(inputs["b2"], np.float32)
    b3 = np.asarray(inputs["b3"], np.float32)
    b4 = np.asarray(inputs["b4"], np.float32)

    EPS = np.float64(1e-5)

    def gv(i):
        return (np.asarray(inputs[f"g{i}"], np.float32),
                np.asarray(inputs[f"be{i}"], np.float32),
                np.asarray(inputs[f"m{i}"], np.float32),
                np.asarray(inputs[f"v{i}"], np.float32))

    g1, be1, m1, v1 = gv(1)
    g2, be2, m2, v2 = gv(2)
    g3, be3, m3, v3 = gv(3)
    # sign(bn(h)) == sign(h + (b - m)) requires gamma > 0 and beta == 0
    assert np.all(g1 > 0) and np.all(be1 == 0), "unsupported bn1 params"
    assert np.all(g2 > 0) and np.all(be2 == 0), "unsupported bn2 params"

    bias1 = (b1 - m1).astype(np.float32)
    bias2 = (b2 - m2).astype(np.float32)
    r3 = 1.0 / np.sqrt(v3.astype(np.float64) + EPS)
    sc3 = (r3 * g3).astype(np.float32)
    sh3 = ((b3 - m3).astype(np.float64) * r3 * g3 + be3).astype(np.float32)

    def wlay(w, kc, dt):  # [out, in] -> [m, k, 128p(in), 128c(out)]
        st = np.sign(w).T.astype(np.float32)            # [in, out]
        kin = kc * 128
        if st.shape[0] < kin:
            st = np.pad(st, ((0, kin - st.shape[0]), (0, 0)))
        mo = st.shape[1] // 128
        return np.ascontiguousarray(
            st.reshape(kc, 128, mo, 128).transpose(2, 0, 1, 3)
        ).astype(dt)

    # s1: chunks 0-5 = rows 0-767; chunk 6 = packed tail (rows 768-783
    # replicated at partitions 0-15 and 32-47, matching the x tail chunk)
    s1f = np.sign(w1).T.astype(np.float32)              # [784, 1024]
    s1t = np.zeros((HC, K1C, 128, 128), np.float32)
    body = s1f[:768].reshape(6, 128, HC, 128)
    tail = s1f[768:784].reshape(16, HC, 128)
    for m in range(HC):
        s1t[m, :6] = body[:, :, m]
        s1t[m, 6, 0:16] = tail[:, m]
        s1t[m, 6, 32:48] = tail[:, m]
    s1t = s1t.astype(f16)
    s2t = wlay(w2, HC, f8)
    s3t = wlay(w3, HC, f8)
    w4t = np.ascontiguousarray(w4.T.astype(f16)).reshape(HC, 128, OUT_F)

    shared = dict(s1t=s1t, s2t=s2t, s3t=s3t, w4t=w4t,
                  bias1=bias1, bias2=bias2, sc3=sc3, sh3=sh3, b4=b4)
    in_maps = []
    for c in range(N_CORES):
        xs = x[c * BS:(c + 1) * BS]                     # [2048, 784]
        xt = xs.T                                       # [784, 2048]
        xhi = xt.astype(f16)
        xlo = (xt - xhi.astype(np.float32)).astype(f16)
        xc = np.zeros((XC, 128, BS), f16)
        xc[0:6] = xhi[:768].reshape(6, 128, BS)
        xc[6:12] = xlo[:768].reshape(6, 128, BS)
        xc[12, 0:16] = xhi[768:784]
        xc[12, 32:48] = xlo[768:784]
        m = dict(shared)
        m["xc"] = np.ascontiguousarray(xc)
        in_maps.append(m)
    return in_maps


def kernel(**inputs):
    global LAST_RESULT
    from concourse.bass_utils import run_bass_kernel_spmd

    if "nc" not in _PLAN:
        _PLAN["nc"] = _build_nc()
    nc = _PLAN["nc"]

    in_maps = _host_prep(inputs)
    br = run_bass_kernel_spmd(
        nc, in_maps, list(range(N_CORES)),
        tmpdir=os.environ.get("KERNEL_TMPDIR") or None,
    )
    LAST_RESULT = br
    out = np.concatenate([br.results[c]["y"] for c in range(N_CORES)], axis=0)
    return out.astype(np.float32)
